# revision 19
# baseline (speedup 1.0000x reference)
"""Trainium2 Bass kernel for nn_NestedFormula.

Tree: DEPTH=4, V=4. Level sizes n4=1, n3=5, n2=25, n1=125, n0=125.
  f1[n] = sum_v lam1[n,v] * x_v^pow1[n,v] + lam0[n]
  fd[n] = sum_v lamd[n,v] * x_v^powd[n,v] * f_{d-1}[5n+v] + f_{d-1}[5n+4]
  out   = f4[0]                          (per batch element)

Strategy (pure data parallel over batch, 8 cores x 16384):
  - ACT (scalar engine) is the hard floor: ~630 exps per batch element at
    1 elem/cycle/lane (1.2 GHz); cost scales with free-dim length only, so
    every activation runs 128 partitions wide with maximal free dim. The
    exp stream (~85us) runs gapless; everything else hides under it.
  - x^p = exp(p * ln x): one packed Ln [128,512]; per-level Exp calls with
    per-partition scale vectors. Level-1 passthrough/lam0 handled by
    exp(0)=1 rows feeding weight columns of block-diagonal G matmuls.
  - ln(x) bounced to DRAM once (fp16), then broadcast-read into the
    replicated layouts (SBUF source APs cannot broadcast partitions).
  - All e-tiles, replicas and G weights are fp16; PSUM accumulates fp32.
  - CHUNK ORDER [3, 0, 1, 2]: chunk 3 first (head-split 1024/3072 so the
    first exp starts as soon as a quarter of its replica lands). Its
    levels-3/4 "late path" (x3b -> ps3b -> x4b -> ps4b -> outb, all
    reading only l3x block [64:96] / l4x rows [32:37]) drains mid-stream
    in the early windows' slack. Chunks 0-2 fuse their f2 drain into l3x
    (blocks [c0,c1,c2] at rows 0/32/96); the "early path" (ps3a with
    zero weights on c3's rows -> x4a -> ps4a -> outa) rides chunk 2's
    (quartered) window, so only the final quarter's slim chain trails
    the stream. Output row DMAs fire piecewise as copies land.
  - Level-4 exp runs packed [80,1024] and unpacks via a DRAM bounce.
  - PE base-partition rules (0/32/64 only; non-zero bases span <= 32
    partitions) force the l3x block order and the zero-padded early
    stationary; ps3+ps4 share one PSUM bank (f3 at partitions 0..,
    f4 written at partition 32+). PSUM: ps1 x3, ps2/ps4a x3, ps3 x2.
  - Engine queues are FROZEN by the Tile scheduler from simulated ready
    times (heap keyed on ready time, then emission priority): DMA queue
    ORDER therefore controls where exps land in the ACT stream, and
    per-piece work is emitted in skewed wavefronts (stage s of piece
    group g-s) so no strict-FIFO queue ever stalls at its head.
  - One preloaded ACT table set (natural_log_exp) serves both Ln and Exp.
"""
import numpy as np

import concourse.bacc as bacc
import concourse.mybir as mybir
from concourse.tile import TileContext

DEPTH = 4
V = 4
B = 131072
M_CORES = 8
BS = B // M_CORES          # 16384 per core
CHUNK = 4096
NCH = BS // CHUNK          # 4
MMN = 512                  # matmul free dim (one PSUM bank)

F32 = mybir.dt.float32
F16 = mybir.dt.float16

N1, N2, N3, N4 = 125, 25, 5, 1
NT1 = 4                    # level-1 j-tiles of 128


def _sigma1(m):
    # psum1 row m -> level-1 node index
    if m < 100:
        return 5 * (m // 4) + (m % 4)
    return 5 * (m - 100) + 4


def _tau2(m):
    # psum2 row m -> level-2 node index
    if m < 20:
        return 5 * (m // 4) + (m % 4)
    return 5 * (m - 20) + 4


def build_constants(lam0, lam1, pow1, lam2, pow2, lam3, pow3, lam4, pow4):
    c = {}
    # ---- level 1: 4 j-tiles of K=128, M=128 (125 used cols) ----
    sc1 = np.zeros((128, NT1), np.float32)
    g1 = np.zeros((NT1, 128, 128), np.float32)
    for n in range(N1):
        for v in range(V):
            j = 4 * n + v
            t, r = divmod(j, 128)
            sc1[r, t] = pow1[n, v]
    for m in range(125):
        n = _sigma1(m)
        for v in range(V):
            j = 4 * n + v
            t, r = divmod(j, 128)
            g1[t, r, m] = lam1[n, v]
        g1[3, 116, m] = lam0[n]          # ones-row (sc1[116,3]=0 -> exp=1)
    c["sc1"] = sc1
    c["g1"] = np.ascontiguousarray(g1.transpose(1, 0, 2).reshape(128, NT1 * 128))

    # ---- level 2: K=128 (100 exp rows + 25 passthrough), M=32 (25 used) ----
    sc2 = np.zeros((128, 1), np.float32)
    g2 = np.zeros((128, 32), np.float32)
    for n in range(N2):
        for v in range(V):
            sc2[4 * n + v, 0] = pow2[n, v]
    for m in range(25):
        n2t = _tau2(m)
        for v in range(V):
            g2[4 * n2t + v, m] = lam2[n2t, v]
        g2[100 + n2t, m] = 1.0           # + f1[5*n2t+4] passthrough
    c["sc2"] = sc2
    c["g2"] = g2

    # ---- level 3 (chunk-packed): rows 32c+m2, cols 5c+u ----
    sc3 = np.zeros((128, 1), np.float32)
    g3 = np.zeros((128, 32), np.float32)
    for cc in range(NCH):
        for m2 in range(25):
            r = 32 * cc + m2
            if m2 < 20:
                n3, v3 = divmod(m2, 4)
                sc3[r, 0] = pow3[n3, v3]
                g3[r, 5 * cc + n3] = lam3[n3, v3]
            else:
                g3[r, 5 * cc + (m2 - 20)] = 1.0   # + f2[5*n3+4]
    c["sc3"] = sc3

    # ---- level 4 (chunk-packed): rows 5c+u (20 rows), cols c ----
    sc4 = np.zeros((20, 1), np.float32)
    g4 = np.zeros((20, NCH), np.float32)
    for cc in range(NCH):
        for u in range(4):
            sc4[5 * cc + u, 0] = pow4[0, u]
            g4[5 * cc + u, cc] = lam4[0, u]
        g4[5 * cc + 4, cc] = 1.0                  # + f3[4]
    c["sc4"] = sc4

    # pack: one scales tensor + one weights tensor (fewer DMA triggers)
    scs = np.zeros((128, 7), np.float32)
    scs[:, 0:4] = c["sc1"]
    scs[:, 4:5] = c["sc2"]
    scs[:, 5:6] = c["sc3"]
    scs[0:80, 6:7] = np.repeat(c["sc4"], 4, axis=0)
    gs = np.zeros((128, 624), np.float32)
    gs[:, 0:512] = c["g1"]
    gs[:, 512:544] = c["g2"]
    # early path: chunks 0-2 only (stationary partitions 0..95)
    gs[0:96, 544:576] = g3[0:96, :]
    # late path: chunk 3 (stationary partitions 96..127, f3 -> cols 0..4)
    gs[96:128, 576:581] = g3[96:128, 15:20]
    gs[0:15, 608:612] = g4[0:15, :]      # early: chunks 0-2 (col 3 -> 0)
    gs[32:37, 612:613] = g4[15:20, 3:4]  # late: chunk 3 -> col 0
    return {"scs": scs, "gs": gs.astype(np.float16)}


def build_bass():
    nc = bacc.Bacc()
    xt = nc.dram_tensor("xt", (V, BS), F32, kind="ExternalInput")
    scs = nc.dram_tensor("scs", (128, 7), F32, kind="ExternalInput")
    gs = nc.dram_tensor("gs", (128, 624), F16, kind="ExternalInput")
    y = nc.dram_tensor("y", (BS,), F32, kind="ExternalOutput")

    EXP = mybir.ActivationFunctionType.Exp
    LN = mybir.ActivationFunctionType.Ln

    with TileContext(nc) as tc:
        with tc.tile_pool(name="const", bufs=1) as cpool, \
             tc.tile_pool(name="dram", bufs=1, space="DRAM") as dpool, \
             tc.tile_pool(name="big", bufs=1) as bpool, \
             tc.tile_pool(name="psum", bufs=2, space="PSUM") as ppool:

            # ---------- x in first: it gates ln and the whole exp chain ---
            # row 4g+v, col i  <->  x[v, 512g+i]
            xc = cpool.tile([128, 512], F32, tag="xc")
            nc.sync.dma_start(
                out=xc[:], in_=xt[:, :].rearrange("v (g i) -> g v i", i=512))

            # ---------- constants into SBUF (packed, 2 triggers) ----------
            sct = cpool.tile([128, 7], F32, tag="sct")
            nc.sync.dma_start(out=sct[:], in_=scs[:, :])
            sct1, sct2, sct3 = sct[:, 0:4], sct[:, 4:5], sct[:, 5:6]
            sct4 = sct[0:80, 6:7]

            # preload the one ACT table set holding BOTH ln and exp, so the
            # compiler's per-function pass doesn't emit two separate loads
            nc.scalar.add_instruction(mybir.InstLoadActFuncSet(
                name=nc.get_next_instruction_name(), act_func_set_id=6,
                ins=[], outs=[]))

            # ---------- ln(x): one packed [128,512] call, fp16 out ---------
            lc = cpool.tile([128, 512], F16, tag="lc")
            nc.scalar.activation(lc[:], xc[:], LN)
            # bounce to DRAM for broadcast reads; trigger from the ACT ring
            # so it issues the moment ln retires. Chunk-0 piece first so the
            # first lrep0 quarter can start immediately.
            ld = dpool.tile([V, BS], F16, tag="ld")
            nc.scalar.dma_start(
                out=ld[:, 0:CHUNK].rearrange("v (g i) -> g v i", i=512),
                in_=lc[0:32, :])
            nc.scalar.dma_start(
                out=ld[:, CHUNK:].rearrange("v (g i) -> g v i", i=512),
                in_=lc[32:128, :])

            # ---------- lrep0 quarter first: it gates the first exp --------
            # (HWDGE ring is FIFO per issuing engine — queue order matters)
            lreps = [None] * NCH
            lrep0 = bpool.tile([128, CHUNK], F16, tag="lrep", bufs=4,
                               name="lrep0")
            lreps[0] = lrep0
            nc.sync.dma_start(
                out=lrep0[:, 0:1024],
                in_=ld[:, 0:1024].unsqueeze(0).broadcast_to([32, V, 1024]))
            nc.sync.dma_start(
                out=lrep0[:, 1024:CHUNK],
                in_=ld[:, 1024:CHUNK].unsqueeze(0)
                    .broadcast_to([32, V, CHUNK - 1024]))

            def load_lrep(cc):
                lrep = bpool.tile([128, CHUNK], F16, tag="lrep", bufs=4,
                                  name=f"lrep{cc}")
                nc.sync.dma_start(
                    out=lrep[:],
                    in_=ld[:, cc * CHUNK:(cc + 1) * CHUNK].unsqueeze(0)
                        .broadcast_to([32, V, CHUNK]))
                lreps[cc] = lrep

            # G weights (first matmul needs them ~23us in)
            gt = cpool.tile([128, 624], F16, tag="gt")
            nc.sync.dma_start(out=gt[:], in_=gs[:, :])
            g1t = gt[:, 0:512]
            g2t = gt[:, 512:544]
            g3at = gt[:, 544:576]
            g3bt = gt[64:96, 576:581]
            g4at = gt[0:15, 608:612]
            g4bt = gt[32:37, 612:613]

            # ---------- phase-B exp inputs, built DIRECTLY from ld -------
            # (no ld8 intermediate: a 32-partition 1MB hop is slow and
            # serializes the sync DMA queue)
            # l3x row blocks [c0, c1, c3, c2]; rows 32b+4q+v = lnx[v] over
            # that block's chunk. Pad rows (20..31 per block) get a memset
            # on the otherwise-idle GpSimd so the exp input is finite.
            l4x = bpool.tile([37, CHUNK], F16, tag="l4x", bufs=1)
            e4pk = bpool.tile([80, 1024], F16, tag="e4pk", bufs=1)
            e4d = dpool.tile([80, 1024], F16, tag="e4d")
            for c4 in range(4):
                nc.scalar.dma_start(
                    out=e4pk[20 * c4:20 * c4 + 16, :],
                    in_=ld[:, c4 * CHUNK:(c4 + 1) * CHUNK]
                        .rearrange("u (b j) -> u b j", j=1024))
                nc.scalar.dma_start(
                    out=e4pk[20 * c4 + 16:20 * c4 + 20, :],
                    in_=ld[0, c4 * CHUNK:(c4 + 1) * CHUNK]
                        .rearrange("(b j) -> b j", j=1024))

            # e4pk row 20c+4u+b, col j = lnx[u] at batch c*4096+1024b+j
            # (u=4 passthrough rows use lnx[0] as a finite filler; scale 0).
            # Packed [80,1024]: ACT cost is per free-dim element, so 4x
            # cheaper than [20,4096]. Unpacks into l4x via a DRAM bounce.
            # l4x rows 0..14: chunks 0-2; rows 32..36: chunk 3 (32-aligned).
            l3x = bpool.tile([128, CHUNK], F16, tag="l3x", bufs=1)
            nc.gpsimd.memset(l3x[:, :], 0.0)
            for b, ch in enumerate([0, 1, 3, 2]):
                nc.scalar.dma_start(
                    out=l3x[32 * b:32 * b + 20, :],
                    in_=ld[:, ch * CHUNK:(ch + 1) * CHUNK].unsqueeze(0)
                        .broadcast_to([5, V, CHUNK]))
            load_lrep(0)
            load_lrep(1)
            load_lrep(2)

            e2s = [None] * NCH
            PW = 512                   # pipeline piece width (1 PSUM bank)

            # ---------- pipeline stages for one column piece ----------
            def st_ps1(cc, pc, e1s, w=PW):
                ps1 = ppool.tile([128, PW], F32, tag="psA", bufs=4,
                                 name="ps1")[:, 0:w]
                for s in range((w + MMN - 1) // MMN):
                    scol = s * MMN
                    sw = min(MMN, w - scol)
                    for t in range(NT1):
                        nc.tensor.matmul(
                            ps1[:, scol:scol + sw],
                            g1t[:, 128 * t:128 * (t + 1)],
                            e1s[t][:, pc + scol:pc + scol + sw],
                            start=(t == 0), stop=(t == NT1 - 1))
                return ps1

            def st_x2(cc, pc, ps1, w=PW):
                e2 = e2s[cc]
                nc.vector.tensor_mul(
                    e2[:, pc:pc + w], e2[:, pc:pc + w], ps1[:])
                ps2 = ppool.tile([32, PW], F32, tag="psB", bufs=4,
                                 name="ps2")[:, 0:w]
                nc.tensor.matmul(
                    ps2[:, 0:w], g2t[:], e2[:, pc:pc + w],
                    start=True, stop=True)
                return ps2

            f2e = bpool.tile([32, CHUNK], F16, tag="f2e", bufs=1)

            def st_x3(cc, pc, ps2, w=PW):
                # X3 = E3 * f2
                if cc == 0:
                    # chunk 0: the l3x exp may not have landed yet (its DMA
                    # chain is long); drain f2 to SBUF so ps2 slots recycle
                    # immediately, and multiply later (see cc==2)
                    nc.vector.tensor_copy(f2e[:, pc:pc + w], ps2[:])
                else:
                    # fused drain of ps2 (in place into l3x rows)
                    nc.vector.tensor_mul(
                        l3x[32 * cc:32 * cc + 32, pc:pc + w],
                        l3x[32 * cc:32 * cc + 32, pc:pc + w], ps2[:])

            def early_tail(pc, w=PW):
                """Levels 3+4 for chunks 0-2 (l3x rows <96, l4x rows <15,
                out rows 0..2) — runs during the chunk-2 window."""
                ps3a = ppool.tile([32, PW], F32, tag="psA", bufs=4,
                                  name="ps3a")[:, 0:w]
                nc.tensor.matmul(ps3a[:, 0:w], g3at[:], l3x[:, pc:pc + w],
                                 start=True, stop=True)
                nc.vector.tensor_mul(l4x[0:15, pc:pc + w],
                                     l4x[0:15, pc:pc + w], ps3a[0:15, :])
                ps4a = ppool.tile([NCH, PW], F32, tag="psB", bufs=4,
                                  name="ps4a")[:, 0:w]
                nc.tensor.matmul(ps4a[:, 0:w], g4at[:], l4x[0:15, pc:pc + w],
                                 start=True, stop=True)
                nc.vector.tensor_copy(outsbA[0:3, pc:pc + w], ps4a[0:3, :])

            def st_ps3b(pc, w=PW):
                ps3b = ppool.tile([32, PW], F32, tag="psA", bufs=4,
                                  name="ps3b")[:, 0:w]
                nc.tensor.matmul(ps3b[:, 0:w], g3bt[:], l3x[:, pc:pc + w],
                                 start=True, stop=True)
                return ps3b

            def st_x4b(pc, ps3b, w=PW):
                nc.vector.tensor_mul(l4x[32:37, pc:pc + w],
                                     l4x[32:37, pc:pc + w], ps3b[0:5, :])
                ps4b = ppool.tile([1, PW], F32, tag="psB", bufs=4,
                                  name="ps4b")[:, 0:w]
                nc.tensor.matmul(ps4b[:, 0:w], g4bt[:], l4x[32:37, pc:pc + w],
                                 start=True, stop=True)
                return ps4b

            def st_out(pc, ps4b, tail, w=PW):
                if tail:
                    nc.scalar.copy(outsbB[0:1, pc:pc + w], ps4b[0:1, :])
                else:
                    nc.vector.tensor_copy(outsbB[0:1, pc:pc + w],
                                          ps4b[0:1, :])

            def phase12(cc, pc, e1s):
                """levels 1+2 for chunk cc, columns [pc, pc+PW)."""
                ps1 = st_ps1(cc, pc, e1s)
                ps2 = st_x2(cc, pc, ps1)
                st_x3(cc, pc, ps2)

            def tail_pipeline(pcs, e1s, cc):
                """Chunk-3 pieces (pc, w) pairs, DVE ops emitted in skewed
                wavefronts so the strict-FIFO vector queue never stalls at
                its head. Only the LATE (chunk-3) levels 3/4 run here."""
                n = len(pcs)
                ps1s = [st_ps1(cc, pc, e1s, w) for pc, w in pcs]
                ps2 = {}
                ps3 = {}
                ps4 = {}
                for d in range(n + 3):
                    for i, (pc, w) in reversed(list(enumerate(pcs))):
                        s = d - i
                        if s == 0:
                            ps2[i] = st_x2(cc, pc, ps1s[i], w)
                        elif s == 1:
                            # x3b: fused drain of ps2 into chunk-3 rows
                            nc.vector.tensor_mul(
                                l3x[96:128, pc:pc + w],
                                l3x[96:128, pc:pc + w], ps2[i][:, :])
                            ps3[i] = st_ps3b(pc, w)
                        elif s == 2:
                            ps4[i] = st_x4b(pc, ps3[i], w)
                        elif s == 3:
                            st_out(pc, ps4[i], tail=(pc >= 3072), w=w)

            outsbA = bpool.tile([3, CHUNK], F32, tag="outsbA", bufs=1)
            outsbB = bpool.tile([1, CHUNK], F32, tag="outsbB", bufs=1)

            # ---------- per-chunk: exps then levels 1+2 ----------
            # chunk 0: head split (1024/3072) so the first exp starts as
            # soon as the first lrep0 quarter lands; chunks 1-2 whole-chunk
            # exp calls; chunk 3: quarter-chunk calls with the late path
            # interleaved so the post-exp tail is short
            for cc in range(NCH):
                if cc == 2:
                    # deferred l3 mul for chunk 0 (fast fp16 SBUF mul)
                    nc.vector.tensor_mul(l3x[0:32, :], l3x[0:32, :],
                                         f2e[:, :])
                last = cc == NCH - 1
                e1s = [None] * NT1
                if cc == 0:
                    splits = [(0, 1024), (1024, CHUNK - 1024)]
                elif last:
                    splits = [(q * 1024, 1024) for q in range(4)]
                else:
                    splits = [(0, CHUNK)]
                for hh, (hc, w) in enumerate(splits):
                    for t in range(NT1):
                        if hh == 0:
                            e1s[t] = bpool.tile([128, CHUNK], F16, tag="e1",
                                                bufs=8, name=f"e1_{cc}_{t}")
                        nc.scalar.activation(
                            e1s[t][:, hc:hc + w], lreps[cc][:, hc:hc + w],
                            EXP, scale=sct1[:, t:t + 1])
                    if hh == 0:
                        e2 = bpool.tile([128, CHUNK], F16, tag="e2", bufs=3,
                                        name=f"e2_{cc}")
                        e2s[cc] = e2
                    nc.scalar.activation(e2s[cc][:, hc:hc + w],
                                         lreps[cc][:, hc:hc + w], EXP,
                                         scale=sct2[:, 0:1])
                    if last:
                        pieces = [(p * PW, PW) for p in
                                  range(hc // PW, (hc + w) // PW)]
                        tail_pipeline(pieces, e1s, cc)
                    else:
                        for p in range(hc // PW, (hc + w) // PW):
                            phase12(cc, p * PW, e1s)
                            if cc == 2:
                                early_tail(p * PW)
                if cc == 0:
                    # hoisted phase-B exps at the end of chunk 0 (their DMA
                    # chains have landed by now)
                    nc.scalar.activation(l3x[:], l3x[:], EXP,
                                         scale=sct3[:, 0:1])
                    nc.scalar.activation(e4pk[:], e4pk[:], EXP,
                                         scale=sct4[:, 0:1])
                    # unpack [80,1024] -> [20,4096] via DRAM bounce
                    # (partition-split SBUF source APs are unsupported)
                    nc.sync.dma_start(out=e4d[:, :], in_=e4pk[:, :])
                    nc.sync.dma_start(
                        out=l4x[0:15, :],
                        in_=e4d[0:60, :].rearrange("(m b) j -> m b j", b=4))
                    nc.sync.dma_start(
                        out=l4x[32:37, :],
                        in_=e4d[60:80, :].rearrange("(m b) j -> m b j", b=4))
                if cc == 2:
                    # out rows 0..2 complete once all early_tail copies land;
                    # this DMA fires mid-chunk-3, off the critical path
                    nc.sync.dma_start(
                        out=y[0:3 * CHUNK].rearrange("(c i) -> c i",
                                                     i=CHUNK),
                        in_=outsbA[:, :])

            nc.sync.dma_start(
                out=y[3 * CHUNK:4 * CHUNK].rearrange("(c i) -> c i",
                                                     i=CHUNK),
                in_=outsbB[:, :])

    nc.compile()
    return nc


def kernel(x, lam0, lam1, pow1, lam2, pow2, lam3, pow3, lam4, pow4):
    x = np.asarray(x, np.float32)
    consts = build_constants(
        np.asarray(lam0, np.float32), np.asarray(lam1, np.float32),
        np.asarray(pow1, np.float32), np.asarray(lam2, np.float32),
        np.asarray(pow2, np.float32), np.asarray(lam3, np.float32),
        np.asarray(pow3, np.float32), np.asarray(lam4, np.float32),
        np.asarray(pow4, np.float32))

    nc = build_bass()

    in_maps = []
    for k in range(M_CORES):
        shard = x[k * BS:(k + 1) * BS, :]
        m = {"xt": np.ascontiguousarray(shard.T)}
        m.update(consts)
        in_maps.append(m)

    from concourse.bass_utils import run_bass_kernel_spmd
    res = run_bass_kernel_spmd(nc, in_maps, list(range(M_CORES)))
    out = np.concatenate([res.results[k]["y"] for k in range(M_CORES)])
    return out[:, None].astype(np.float32)


if __name__ == "__main__":
    import reference
    inputs = {k: np.asarray(v) for k, v in reference.setup_inputs().items()}
    got = kernel(**inputs)
    exp = np.asarray(reference.reference(**inputs))
    err = np.abs(got - exp).max() / (np.abs(exp).max() + 1e-30)
    print("shape", got.shape, "relerr", err)


# revision 20
# speedup vs baseline: 1.2263x; 1.2263x over previous
"""Trainium2 Bass kernel for nn_NestedFormula.

Tree: DEPTH=4, V=4. Level sizes n4=1, n3=5, n2=25, n1=125, n0=125.
  f1[n] = sum_v lam1[n,v] * x_v^pow1[n,v] + lam0[n]
  fd[n] = sum_v lamd[n,v] * x_v^powd[n,v] * f_{d-1}[5n+v] + f_{d-1}[5n+4]
  out   = f4[0]                          (per batch element)

Strategy (pure data parallel over batch, 8 cores x 16384):
  - ACT (scalar engine) is the hard floor: ~630 exps per batch element at
    1 elem/cycle/lane; cost scales with free-dim length only, so every
    activation runs 128 partitions wide with maximal free dim.
  - x^p = exp(p * ln x): one packed Ln [128,512]; per-level Exp calls with
    per-partition scale vectors. Level-1 passthrough/lam0 handled by
    exp(0)=1 rows feeding weight columns of block-diagonal G matmuls.
  - ln(x) bounced to DRAM once (fp16), broadcast-read into replicated
    layouts with step-0 partition APs. Head is split (1024/3072) so the
    first e1 exp starts as soon as a quarter of lrep0 lands.
  - All e-tiles, replicas and G weights are fp16; PSUM accumulates fp32.
  - Levels 3/4 are split into an EARLY path (chunks 0-2, zero-padded
    g3a/g4a stationaries) that runs during the chunk-2 window, and a LATE
    path (chunk 3, g3b/g4b) so only a slim per-piece chain trails the
    final exp. Output rows 0-2 DMA out mid-stream; row 3 at the end.
  - Level-4 exp runs packed [80,1024] and unpacks via a DRAM bounce.
  - Chunk 3 runs quarter-granular exps with the late path software-
    pipelined in skewed wavefronts; final output copies ride the
    then-idle scalar engine.
  - One preloaded ACT table set (natural_log_exp) serves both Ln and Exp.
"""
import numpy as np

import concourse.bacc as bacc
import concourse.mybir as mybir
from concourse.tile import TileContext

DEPTH = 4
V = 4
B = 131072
M_CORES = 8
BS = B // M_CORES          # 16384 per core
CHUNK = 4096
NCH = BS // CHUNK          # 4
MMN = 512                  # matmul free dim (one PSUM bank)

F32 = mybir.dt.float32
F16 = mybir.dt.float16

N1, N2, N3, N4 = 125, 25, 5, 1
NT1 = 4                    # level-1 j-tiles of 128


def _sigma1(m):
    # psum1 row m -> level-1 node index
    if m < 100:
        return 5 * (m // 4) + (m % 4)
    return 5 * (m - 100) + 4


def _tau2(m):
    # psum2 row m -> level-2 node index
    if m < 20:
        return 5 * (m // 4) + (m % 4)
    return 5 * (m - 20) + 4


def build_constants(lam0, lam1, pow1, lam2, pow2, lam3, pow3, lam4, pow4):
    c = {}
    # ---- level 1: 4 j-tiles of K=128, M=128 (125 used cols) ----
    sc1 = np.zeros((128, NT1), np.float32)
    g1 = np.zeros((NT1, 128, 128), np.float32)
    for n in range(N1):
        for v in range(V):
            j = 4 * n + v
            t, r = divmod(j, 128)
            sc1[r, t] = pow1[n, v]
    for m in range(125):
        n = _sigma1(m)
        for v in range(V):
            j = 4 * n + v
            t, r = divmod(j, 128)
            g1[t, r, m] = lam1[n, v]
        g1[3, 116, m] = lam0[n]          # ones-row (sc1[116,3]=0 -> exp=1)
    c["sc1"] = sc1
    c["g1"] = np.ascontiguousarray(g1.transpose(1, 0, 2).reshape(128, NT1 * 128))

    # ---- level 2: K=128 (100 exp rows + 25 passthrough), M=32 (25 used) ----
    sc2 = np.zeros((128, 1), np.float32)
    g2 = np.zeros((128, 32), np.float32)
    for n in range(N2):
        for v in range(V):
            sc2[4 * n + v, 0] = pow2[n, v]
    for m in range(25):
        n2t = _tau2(m)
        for v in range(V):
            g2[4 * n2t + v, m] = lam2[n2t, v]
        g2[100 + n2t, m] = 1.0           # + f1[5*n2t+4] passthrough
    c["sc2"] = sc2
    c["g2"] = g2

    # ---- level 3 (chunk-packed): rows 32c+m2, cols 5c+u ----
    sc3 = np.zeros((128, 1), np.float32)
    g3 = np.zeros((128, 32), np.float32)
    for cc in range(NCH):
        for m2 in range(25):
            r = 32 * cc + m2
            if m2 < 20:
                n3, v3 = divmod(m2, 4)
                sc3[r, 0] = pow3[n3, v3]
                g3[r, 5 * cc + n3] = lam3[n3, v3]
            else:
                g3[r, 5 * cc + (m2 - 20)] = 1.0   # + f2[5*n3+4]
    c["sc3"] = sc3

    # ---- level 4 (chunk-packed): rows 5c+u (20 rows), cols c ----
    sc4 = np.zeros((20, 1), np.float32)
    g4 = np.zeros((20, NCH), np.float32)
    for cc in range(NCH):
        for u in range(4):
            sc4[5 * cc + u, 0] = pow4[0, u]
            g4[5 * cc + u, cc] = lam4[0, u]
        g4[5 * cc + 4, cc] = 1.0                  # + f3[4]
    c["sc4"] = sc4

    # pack: one scales tensor + one weights tensor (fewer DMA triggers)
    scs = np.zeros((128, 7), np.float32)
    scs[:, 0:4] = c["sc1"]
    scs[:, 4:5] = c["sc2"]
    scs[:, 5:6] = c["sc3"]
    scs[0:80, 6:7] = np.repeat(c["sc4"], 4, axis=0)
    gs = np.zeros((128, 624), np.float32)
    gs[:, 0:512] = c["g1"]
    gs[:, 512:544] = c["g2"]
    # early path: chunks 0-2 only (stationary partitions 0..95)
    gs[0:96, 544:576] = g3[0:96, :]
    # late path: chunk 3 (stationary partitions 96..127, f3 -> cols 0..4)
    gs[96:128, 576:581] = g3[96:128, 15:20]
    gs[0:15, 608:612] = g4[0:15, :]      # early: chunks 0-2 (col 3 -> 0)
    gs[32:37, 612:613] = g4[15:20, 3:4]  # late: chunk 3 -> col 0
    return {"scs": scs, "gs": gs.astype(np.float16)}


def build_bass():
    nc = bacc.Bacc()
    xt = nc.dram_tensor("xt", (V, BS), F32, kind="ExternalInput")
    scs = nc.dram_tensor("scs", (128, 7), F32, kind="ExternalInput")
    gs = nc.dram_tensor("gs", (128, 624), F16, kind="ExternalInput")
    y = nc.dram_tensor("y", (BS,), F32, kind="ExternalOutput")

    EXP = mybir.ActivationFunctionType.Exp
    LN = mybir.ActivationFunctionType.Ln

    with TileContext(nc) as tc:
        with tc.tile_pool(name="const", bufs=1) as cpool, \
             tc.tile_pool(name="dram", bufs=1, space="DRAM") as dpool, \
             tc.tile_pool(name="big", bufs=1) as bpool, \
             tc.tile_pool(name="psum", bufs=2, space="PSUM") as ppool:

            # ---------- x in first: it gates ln and the whole exp chain ---
            # row 4g+v, col i  <->  x[v, 512g+i]
            xc = cpool.tile([128, 512], F32, tag="xc")
            nc.sync.dma_start(
                out=xc[:], in_=xt[:, :].rearrange("v (g i) -> g v i", i=512))

            # ---------- constants into SBUF (packed, 2 triggers) ----------
            sct = cpool.tile([128, 7], F32, tag="sct")
            nc.sync.dma_start(out=sct[:], in_=scs[:, :])
            sct1, sct2, sct3 = sct[:, 0:4], sct[:, 4:5], sct[:, 5:6]
            sct4 = sct[0:80, 6:7]

            # preload the one ACT table set holding BOTH ln and exp, so the
            # compiler's per-function pass doesn't emit two separate loads
            nc.scalar.add_instruction(mybir.InstLoadActFuncSet(
                name=nc.get_next_instruction_name(), act_func_set_id=6,
                ins=[], outs=[]))

            # ---------- ln(x): one packed [128,512] call, fp16 out ---------
            lc = cpool.tile([128, 512], F16, tag="lc")
            nc.scalar.activation(lc[:], xc[:], LN)
            # bounce to DRAM for broadcast reads; trigger from the ACT ring
            # so it issues the moment ln retires. Chunk-0 piece first so the
            # first lrep0 quarter can start immediately.
            ld = dpool.tile([V, BS], F16, tag="ld")
            nc.scalar.dma_start(
                out=ld[:, 0:CHUNK].rearrange("v (g i) -> g v i", i=512),
                in_=lc[0:32, :])
            nc.scalar.dma_start(
                out=ld[:, CHUNK:].rearrange("v (g i) -> g v i", i=512),
                in_=lc[32:128, :])

            # ---------- lrep0 quarter first: it gates the first exp --------
            # (HWDGE ring is FIFO per issuing engine — queue order matters)
            lreps = [None] * NCH
            lrep0 = bpool.tile([128, CHUNK], F16, tag="lrep", bufs=4,
                               name="lrep0")
            lreps[0] = lrep0
            nc.sync.dma_start(
                out=lrep0[:, 0:1024],
                in_=ld[:, 0:1024].unsqueeze(0).broadcast_to([32, V, 1024]))
            nc.sync.dma_start(
                out=lrep0[:, 1024:CHUNK],
                in_=ld[:, 1024:CHUNK].unsqueeze(0)
                    .broadcast_to([32, V, CHUNK - 1024]))

            def load_lrep(cc):
                lrep = bpool.tile([128, CHUNK], F16, tag="lrep", bufs=4,
                                  name=f"lrep{cc}")
                nc.sync.dma_start(
                    out=lrep[:],
                    in_=ld[:, cc * CHUNK:(cc + 1) * CHUNK].unsqueeze(0)
                        .broadcast_to([32, V, CHUNK]))
                lreps[cc] = lrep

            # G weights (first matmul needs them ~23us in)
            gt = cpool.tile([128, 624], F16, tag="gt")
            nc.sync.dma_start(out=gt[:], in_=gs[:, :])
            g1t = gt[:, 0:512]
            g2t = gt[:, 512:544]
            g3at = gt[:, 544:576]
            g3bt = gt[64:96, 576:581]
            g4at = gt[0:15, 608:612]
            g4bt = gt[32:37, 612:613]

            # ---------- phase-B exp inputs, built DIRECTLY from ld -------
            # (no ld8 intermediate: a 32-partition 1MB hop is slow and
            # serializes the sync DMA queue)
            # l3x row blocks [c0, c1, c3, c2]; rows 32b+4q+v = lnx[v] over
            # that block's chunk. Pad rows (20..31 per block) get a memset
            # on the otherwise-idle GpSimd so the exp input is finite.
            l4x = bpool.tile([37, CHUNK], F16, tag="l4x", bufs=1)
            e4pk = bpool.tile([80, 1024], F16, tag="e4pk", bufs=1)
            e4d = dpool.tile([80, 1024], F16, tag="e4d")
            for c4 in range(4):
                nc.scalar.dma_start(
                    out=e4pk[20 * c4:20 * c4 + 16, :],
                    in_=ld[:, c4 * CHUNK:(c4 + 1) * CHUNK]
                        .rearrange("u (b j) -> u b j", j=1024))
                nc.scalar.dma_start(
                    out=e4pk[20 * c4 + 16:20 * c4 + 20, :],
                    in_=ld[0, c4 * CHUNK:(c4 + 1) * CHUNK]
                        .rearrange("(b j) -> b j", j=1024))

            # e4pk row 20c+4u+b, col j = lnx[u] at batch c*4096+1024b+j
            # (u=4 passthrough rows use lnx[0] as a finite filler; scale 0).
            # Packed [80,1024]: ACT cost is per free-dim element, so 4x
            # cheaper than [20,4096]. Unpacks into l4x via a DRAM bounce.
            # l4x rows 0..14: chunks 0-2; rows 32..36: chunk 3 (32-aligned).
            l3x = bpool.tile([128, CHUNK], F16, tag="l3x", bufs=1)
            nc.gpsimd.memset(l3x[:, :], 0.0)
            for b, ch in enumerate([0, 1, 3, 2]):
                nc.scalar.dma_start(
                    out=l3x[32 * b:32 * b + 20, :],
                    in_=ld[:, ch * CHUNK:(ch + 1) * CHUNK].unsqueeze(0)
                        .broadcast_to([5, V, CHUNK]))
            load_lrep(0)
            load_lrep(1)
            load_lrep(2)

            e2s = [None] * NCH
            PW = 512                   # pipeline piece width (1 PSUM bank)

            # ---------- pipeline stages for one column piece ----------
            def st_ps1(cc, pc, e1s, w=PW):
                ps1 = ppool.tile([128, PW], F32, tag="psA", bufs=4,
                                 name="ps1")[:, 0:w]
                for s in range((w + MMN - 1) // MMN):
                    scol = s * MMN
                    sw = min(MMN, w - scol)
                    for t in range(NT1):
                        nc.tensor.matmul(
                            ps1[:, scol:scol + sw],
                            g1t[:, 128 * t:128 * (t + 1)],
                            e1s[t][:, pc + scol:pc + scol + sw],
                            start=(t == 0), stop=(t == NT1 - 1))
                return ps1

            def st_x2(cc, pc, ps1, w=PW):
                e2 = e2s[cc]
                nc.vector.tensor_mul(
                    e2[:, pc:pc + w], e2[:, pc:pc + w], ps1[:])
                ps2 = ppool.tile([32, PW], F32, tag="psB", bufs=4,
                                 name="ps2")[:, 0:w]
                nc.tensor.matmul(
                    ps2[:, 0:w], g2t[:], e2[:, pc:pc + w],
                    start=True, stop=True)
                return ps2

            f2e = bpool.tile([32, CHUNK], F16, tag="f2e", bufs=1)

            def st_x3(cc, pc, ps2, w=PW):
                # X3 = E3 * f2
                if cc == 0:
                    # chunk 0: the l3x exp may not have landed yet (its DMA
                    # chain is long); drain f2 to SBUF so ps2 slots recycle
                    # immediately, and multiply later (see cc==2)
                    nc.vector.tensor_copy(f2e[:, pc:pc + w], ps2[:])
                else:
                    # fused drain of ps2 (in place into l3x rows)
                    nc.vector.tensor_mul(
                        l3x[32 * cc:32 * cc + 32, pc:pc + w],
                        l3x[32 * cc:32 * cc + 32, pc:pc + w], ps2[:])

            def early_tail(pc, w=PW):
                """Levels 3+4 for chunks 0-2 (l3x rows <96, l4x rows <15,
                out rows 0..2) — runs during the chunk-2 window."""
                ps3a = ppool.tile([32, PW], F32, tag="psA", bufs=4,
                                  name="ps3a")[:, 0:w]
                nc.tensor.matmul(ps3a[:, 0:w], g3at[:], l3x[:, pc:pc + w],
                                 start=True, stop=True)
                nc.vector.tensor_mul(l4x[0:15, pc:pc + w],
                                     l4x[0:15, pc:pc + w], ps3a[0:15, :])
                ps4a = ppool.tile([NCH, PW], F32, tag="psB", bufs=4,
                                  name="ps4a")[:, 0:w]
                nc.tensor.matmul(ps4a[:, 0:w], g4at[:], l4x[0:15, pc:pc + w],
                                 start=True, stop=True)
                nc.vector.tensor_copy(outsbA[0:3, pc:pc + w], ps4a[0:3, :])

            def st_ps3b(pc, w=PW):
                ps3b = ppool.tile([32, PW], F32, tag="psA", bufs=4,
                                  name="ps3b")[:, 0:w]
                nc.tensor.matmul(ps3b[:, 0:w], g3bt[:], l3x[:, pc:pc + w],
                                 start=True, stop=True)
                return ps3b

            def st_x4b(pc, ps3b, w=PW):
                nc.vector.tensor_mul(l4x[32:37, pc:pc + w],
                                     l4x[32:37, pc:pc + w], ps3b[0:5, :])
                ps4b = ppool.tile([1, PW], F32, tag="psB", bufs=4,
                                  name="ps4b")[:, 0:w]
                nc.tensor.matmul(ps4b[:, 0:w], g4bt[:], l4x[32:37, pc:pc + w],
                                 start=True, stop=True)
                return ps4b

            def st_out(pc, ps4b, tail, w=PW):
                if tail:
                    nc.scalar.copy(outsbB[0:1, pc:pc + w], ps4b[0:1, :])
                else:
                    nc.vector.tensor_copy(outsbB[0:1, pc:pc + w],
                                          ps4b[0:1, :])

            def phase12(cc, pc, e1s):
                """levels 1+2 for chunk cc, columns [pc, pc+PW)."""
                ps1 = st_ps1(cc, pc, e1s)
                ps2 = st_x2(cc, pc, ps1)
                st_x3(cc, pc, ps2)

            def tail_pipeline(pcs, e1s, cc):
                """Chunk-3 pieces (pc, w) pairs, DVE ops emitted in skewed
                wavefronts so the strict-FIFO vector queue never stalls at
                its head. Only the LATE (chunk-3) levels 3/4 run here."""
                n = len(pcs)
                ps1s = [st_ps1(cc, pc, e1s, w) for pc, w in pcs]
                ps2 = {}
                ps3 = {}
                ps4 = {}
                for d in range(n + 3):
                    for i, (pc, w) in reversed(list(enumerate(pcs))):
                        s = d - i
                        if s == 0:
                            ps2[i] = st_x2(cc, pc, ps1s[i], w)
                        elif s == 1:
                            # x3b: fused drain of ps2 into chunk-3 rows
                            nc.vector.tensor_mul(
                                l3x[96:128, pc:pc + w],
                                l3x[96:128, pc:pc + w], ps2[i][:, :])
                            ps3[i] = st_ps3b(pc, w)
                        elif s == 2:
                            ps4[i] = st_x4b(pc, ps3[i], w)
                        elif s == 3:
                            st_out(pc, ps4[i], tail=(pc >= 3072), w=w)

            outsbA = bpool.tile([3, CHUNK], F32, tag="outsbA", bufs=1)
            outsbB = bpool.tile([1, CHUNK], F32, tag="outsbB", bufs=1)

            # ---------- per-chunk: exps then levels 1+2 ----------
            # chunk 0: head split (1024/3072) so the first exp starts as
            # soon as the first lrep0 quarter lands; chunks 1-2 whole-chunk
            # exp calls; chunk 3: quarter-chunk calls with the late path
            # interleaved so the post-exp tail is short
            for cc in range(NCH):
                if cc == 2:
                    # deferred l3 mul for chunk 0 (fast fp16 SBUF mul)
                    nc.vector.tensor_mul(l3x[0:32, :], l3x[0:32, :],
                                         f2e[:, :])
                last = cc == NCH - 1
                e1s = [None] * NT1
                if cc == 0:
                    splits = [(0, 1024), (1024, CHUNK - 1024)]
                elif last:
                    splits = [(q * 1024, 1024) for q in range(4)]
                else:
                    splits = [(0, CHUNK)]
                for hh, (hc, w) in enumerate(splits):
                    for t in range(NT1):
                        if hh == 0:
                            e1s[t] = bpool.tile([128, CHUNK], F16, tag="e1",
                                                bufs=8, name=f"e1_{cc}_{t}")
                        nc.scalar.activation(
                            e1s[t][:, hc:hc + w], lreps[cc][:, hc:hc + w],
                            EXP, scale=sct1[:, t:t + 1])
                    if hh == 0:
                        e2 = bpool.tile([128, CHUNK], F16, tag="e2", bufs=3,
                                        name=f"e2_{cc}")
                        e2s[cc] = e2
                    nc.scalar.activation(e2s[cc][:, hc:hc + w],
                                         lreps[cc][:, hc:hc + w], EXP,
                                         scale=sct2[:, 0:1])
                    if last:
                        pieces = [(p * PW, PW) for p in
                                  range(hc // PW, (hc + w) // PW)]
                        tail_pipeline(pieces, e1s, cc)
                    else:
                        for p in range(hc // PW, (hc + w) // PW):
                            phase12(cc, p * PW, e1s)
                            if cc == 2:
                                early_tail(p * PW)
                if cc == 0:
                    # hoisted phase-B exps at the end of chunk 0 (their DMA
                    # chains have landed by now)
                    nc.scalar.activation(l3x[:], l3x[:], EXP,
                                         scale=sct3[:, 0:1])
                    nc.scalar.activation(e4pk[:], e4pk[:], EXP,
                                         scale=sct4[:, 0:1])
                    # unpack [80,1024] -> [20,4096] via DRAM bounce
                    # (partition-split SBUF source APs are unsupported)
                    nc.sync.dma_start(out=e4d[:, :], in_=e4pk[:, :])
                    nc.sync.dma_start(
                        out=l4x[0:15, :],
                        in_=e4d[0:60, :].rearrange("(m b) j -> m b j", b=4))
                    nc.sync.dma_start(
                        out=l4x[32:37, :],
                        in_=e4d[60:80, :].rearrange("(m b) j -> m b j", b=4))
                if cc == 2:
                    # out rows 0..2 complete once all early_tail copies land;
                    # this DMA fires mid-chunk-3, off the critical path
                    nc.sync.dma_start(
                        out=y[0:3 * CHUNK].rearrange("(c i) -> c i",
                                                     i=CHUNK),
                        in_=outsbA[:, :])

            nc.sync.dma_start(
                out=y[3 * CHUNK:4 * CHUNK].rearrange("(c i) -> c i",
                                                     i=CHUNK),
                in_=outsbB[:, :])

    nc.compile()
    return nc


def kernel(x, lam0, lam1, pow1, lam2, pow2, lam3, pow3, lam4, pow4):
    x = np.asarray(x, np.float32)
    consts = build_constants(
        np.asarray(lam0, np.float32), np.asarray(lam1, np.float32),
        np.asarray(pow1, np.float32), np.asarray(lam2, np.float32),
        np.asarray(pow2, np.float32), np.asarray(lam3, np.float32),
        np.asarray(pow3, np.float32), np.asarray(lam4, np.float32),
        np.asarray(pow4, np.float32))

    nc = build_bass()

    in_maps = []
    for k in range(M_CORES):
        shard = x[k * BS:(k + 1) * BS, :]
        m = {"xt": np.ascontiguousarray(shard.T)}
        m.update(consts)
        in_maps.append(m)

    from concourse.bass_utils import run_bass_kernel_spmd
    res = run_bass_kernel_spmd(nc, in_maps, list(range(M_CORES)))
    out = np.concatenate([res.results[k]["y"] for k in range(M_CORES)])
    return out[:, None].astype(np.float32)


if __name__ == "__main__":
    import reference
    inputs = {k: np.asarray(v) for k, v in reference.setup_inputs().items()}
    got = kernel(**inputs)
    exp = np.asarray(reference.reference(**inputs))
    err = np.abs(got - exp).max() / (np.abs(exp).max() + 1e-30)
    print("shape", got.shape, "relerr", err)


# revision 21
# speedup vs baseline: 1.2283x; 1.0017x over previous
"""Trainium2 Bass kernel for nn_NestedFormula.

Tree: DEPTH=4, V=4. Level sizes n4=1, n3=5, n2=25, n1=125, n0=125.
  f1[n] = sum_v lam1[n,v] * x_v^pow1[n,v] + lam0[n]
  fd[n] = sum_v lamd[n,v] * x_v^powd[n,v] * f_{d-1}[5n+v] + f_{d-1}[5n+4]
  out   = f4[0]                          (per batch element)

Strategy (pure data parallel over batch, 8 cores x 16384):
  - ACT (scalar engine) is the hard floor: ~630 exps per batch element at
    1 elem/cycle/lane; cost scales with free-dim length only, so every
    activation runs 128 partitions wide with maximal free dim.
  - x^p = exp(p * ln x): one packed Ln [128,512]; per-level Exp calls with
    per-partition scale vectors. Level-1 passthrough/lam0 handled by
    exp(0)=1 rows feeding weight columns of block-diagonal G matmuls.
  - ln(x) bounced to DRAM once (fp16), broadcast-read into replicated
    layouts with step-0 partition APs. Head is split (1024/3072) so the
    first e1 exp starts as soon as a quarter of lrep0 lands.
  - All e-tiles, replicas and G weights are fp16; PSUM accumulates fp32.
  - Levels 3/4 are split into an EARLY path (chunks 0-2, zero-padded
    g3a/g4a stationaries) that runs during the chunk-2 window, and a LATE
    path (chunk 3, g3b/g4b) so only a slim per-piece chain trails the
    final exp. Output rows 0-2 DMA out mid-stream; row 3 at the end.
  - Level-4 exp runs packed [80,1024] and unpacks via a DRAM bounce.
  - Chunk 3 runs quarter-granular exps with the late path software-
    pipelined in skewed wavefronts; final output copies ride the
    then-idle scalar engine.
  - One preloaded ACT table set (natural_log_exp) serves both Ln and Exp.
"""
import numpy as np

import concourse.bacc as bacc
import concourse.mybir as mybir
from concourse.tile import TileContext

DEPTH = 4
V = 4
B = 131072
M_CORES = 8
BS = B // M_CORES          # 16384 per core
CHUNK = 4096
NCH = BS // CHUNK          # 4
MMN = 512                  # matmul free dim (one PSUM bank)

F32 = mybir.dt.float32
F16 = mybir.dt.float16

N1, N2, N3, N4 = 125, 25, 5, 1
NT1 = 4                    # level-1 j-tiles of 128


def _sigma1(m):
    # psum1 row m -> level-1 node index
    if m < 100:
        return 5 * (m // 4) + (m % 4)
    return 5 * (m - 100) + 4


def _tau2(m):
    # psum2 row m -> level-2 node index
    if m < 20:
        return 5 * (m // 4) + (m % 4)
    return 5 * (m - 20) + 4


def build_constants(lam0, lam1, pow1, lam2, pow2, lam3, pow3, lam4, pow4):
    c = {}
    # ---- level 1: 4 j-tiles of K=128, M=128 (125 used cols) ----
    sc1 = np.zeros((128, NT1), np.float32)
    g1 = np.zeros((NT1, 128, 128), np.float32)
    for n in range(N1):
        for v in range(V):
            j = 4 * n + v
            t, r = divmod(j, 128)
            sc1[r, t] = pow1[n, v]
    for m in range(125):
        n = _sigma1(m)
        for v in range(V):
            j = 4 * n + v
            t, r = divmod(j, 128)
            g1[t, r, m] = lam1[n, v]
        g1[3, 116, m] = lam0[n]          # ones-row (sc1[116,3]=0 -> exp=1)
    c["sc1"] = sc1
    c["g1"] = np.ascontiguousarray(g1.transpose(1, 0, 2).reshape(128, NT1 * 128))

    # ---- level 2: K=128 (100 exp rows + 25 passthrough), M=32 (25 used) ----
    sc2 = np.zeros((128, 1), np.float32)
    g2 = np.zeros((128, 32), np.float32)
    for n in range(N2):
        for v in range(V):
            sc2[4 * n + v, 0] = pow2[n, v]
    for m in range(25):
        n2t = _tau2(m)
        for v in range(V):
            g2[4 * n2t + v, m] = lam2[n2t, v]
        g2[100 + n2t, m] = 1.0           # + f1[5*n2t+4] passthrough
    c["sc2"] = sc2
    c["g2"] = g2

    # ---- level 3 (chunk-packed): rows 32c+m2, cols 5c+u ----
    sc3 = np.zeros((128, 1), np.float32)
    g3 = np.zeros((128, 32), np.float32)
    for cc in range(NCH):
        for m2 in range(25):
            r = 32 * cc + m2
            if m2 < 20:
                n3, v3 = divmod(m2, 4)
                sc3[r, 0] = pow3[n3, v3]
                g3[r, 5 * cc + n3] = lam3[n3, v3]
            else:
                g3[r, 5 * cc + (m2 - 20)] = 1.0   # + f2[5*n3+4]
    c["sc3"] = sc3

    # ---- level 4 (chunk-packed): rows 5c+u (20 rows), cols c ----
    sc4 = np.zeros((20, 1), np.float32)
    g4 = np.zeros((20, NCH), np.float32)
    for cc in range(NCH):
        for u in range(4):
            sc4[5 * cc + u, 0] = pow4[0, u]
            g4[5 * cc + u, cc] = lam4[0, u]
        g4[5 * cc + 4, cc] = 1.0                  # + f3[4]
    c["sc4"] = sc4

    # pack: one scales tensor + one weights tensor (fewer DMA triggers)
    scs = np.zeros((128, 7), np.float32)
    scs[:, 0:4] = c["sc1"]
    scs[:, 4:5] = c["sc2"]
    scs[:, 5:6] = c["sc3"]
    scs[0:80, 6:7] = np.repeat(c["sc4"], 4, axis=0)
    gs = np.zeros((128, 624), np.float32)
    gs[:, 0:512] = c["g1"]
    gs[:, 512:544] = c["g2"]
    # early path: chunks 0-2 only (stationary partitions 0..95)
    gs[0:96, 544:576] = g3[0:96, :]
    # late path: chunk 3 (stationary partitions 96..127, f3 -> cols 0..4)
    gs[96:128, 576:581] = g3[96:128, 15:20]
    gs[0:15, 608:612] = g4[0:15, :]      # early: chunks 0-2 (col 3 -> 0)
    gs[32:37, 612:613] = g4[15:20, 3:4]  # late: chunk 3 -> col 0
    return {"scs": scs, "gs": gs.astype(np.float16)}


def build_bass():
    nc = bacc.Bacc()
    xt = nc.dram_tensor("xt", (V, BS), F32, kind="ExternalInput")
    scs = nc.dram_tensor("scs", (128, 7), F32, kind="ExternalInput")
    gs = nc.dram_tensor("gs", (128, 624), F16, kind="ExternalInput")
    y = nc.dram_tensor("y", (BS,), F32, kind="ExternalOutput")

    EXP = mybir.ActivationFunctionType.Exp
    LN = mybir.ActivationFunctionType.Ln

    with TileContext(nc) as tc:
        with tc.tile_pool(name="const", bufs=1) as cpool, \
             tc.tile_pool(name="dram", bufs=1, space="DRAM") as dpool, \
             tc.tile_pool(name="big", bufs=1) as bpool, \
             tc.tile_pool(name="psum", bufs=2, space="PSUM") as ppool:

            # ---------- x in first: it gates ln and the whole exp chain ---
            # row 4g+v, col i  <->  x[v, 512g+i]
            xc = cpool.tile([128, 512], F32, tag="xc")
            nc.sync.dma_start(
                out=xc[:], in_=xt[:, :].rearrange("v (g i) -> g v i", i=512))

            # ---------- constants into SBUF ----------
            # scales ride the SCALAR ring: one tiny trigger ahead of the
            # table load (absorbed), freeing the sync queue's second slot
            # for the xt broadcast that gates the first exp
            sct = cpool.tile([128, 7], F32, tag="sct")
            nc.scalar.dma_start(out=sct[:], in_=scs[:, :])
            sct1, sct2, sct3 = sct[:, 0:4], sct[:, 4:5], sct[:, 5:6]
            sct4 = sct[0:80, 6:7]

            # preload the one ACT table set holding BOTH ln and exp, so the
            # compiler's per-function pass doesn't emit two separate loads
            nc.scalar.add_instruction(mybir.InstLoadActFuncSet(
                name=nc.get_next_instruction_name(), act_func_set_id=6,
                ins=[], outs=[]))

            # ---------- ln(x): one packed [128,512] call, fp16 out ---------
            lc = cpool.tile([128, 512], F16, tag="lc")
            nc.scalar.activation(lc[:], xc[:], LN)
            # bounce to DRAM for broadcast reads; trigger from the ACT ring
            # so it issues the moment ln retires. Chunk-0 piece first so the
            # first lrep0 quarter can start immediately.
            ld = dpool.tile([V, BS], F16, tag="ld")
            nc.scalar.dma_start(
                out=ld[:, 0:CHUNK].rearrange("v (g i) -> g v i", i=512),
                in_=lc[0:32, :])
            nc.scalar.dma_start(
                out=ld[:, CHUNK:].rearrange("v (g i) -> g v i", i=512),
                in_=lc[32:128, :])

            # ---------- lrep0 quarter first: it gates the first exp --------
            # (HWDGE ring is FIFO per issuing engine — queue order matters)
            lreps = [None] * NCH
            lrep0 = bpool.tile([128, CHUNK], F16, tag="lrep", bufs=4,
                               name="lrep0")
            lreps[0] = lrep0
            nc.sync.dma_start(
                out=lrep0[:, 0:1024],
                in_=ld[:, 0:1024].unsqueeze(0).broadcast_to([32, V, 1024]))
            nc.sync.dma_start(
                out=lrep0[:, 1024:CHUNK],
                in_=ld[:, 1024:CHUNK].unsqueeze(0)
                    .broadcast_to([32, V, CHUNK - 1024]))

            def load_lrep(cc):
                lrep = bpool.tile([128, CHUNK], F16, tag="lrep", bufs=4,
                                  name=f"lrep{cc}")
                nc.sync.dma_start(
                    out=lrep[:],
                    in_=ld[:, cc * CHUNK:(cc + 1) * CHUNK].unsqueeze(0)
                        .broadcast_to([32, V, CHUNK]))
                lreps[cc] = lrep

            # G weights (first matmul needs them ~23us in)
            gt = cpool.tile([128, 624], F16, tag="gt")
            nc.sync.dma_start(out=gt[:], in_=gs[:, :])
            g1t = gt[:, 0:512]
            g2t = gt[:, 512:544]
            g3at = gt[:, 544:576]
            g3bt = gt[64:96, 576:581]
            g4at = gt[0:15, 608:612]
            g4bt = gt[32:37, 612:613]

            # ---------- phase-B exp inputs, built DIRECTLY from ld -------
            # (no ld8 intermediate: a 32-partition 1MB hop is slow and
            # serializes the sync DMA queue)
            # l3x row blocks [c0, c1, c3, c2]; rows 32b+4q+v = lnx[v] over
            # that block's chunk. Pad rows (20..31 per block) get a memset
            # on the otherwise-idle GpSimd so the exp input is finite.
            l4x = bpool.tile([37, CHUNK], F16, tag="l4x", bufs=1)
            e4pk = bpool.tile([80, 1024], F16, tag="e4pk", bufs=1)
            e4d = dpool.tile([80, 1024], F16, tag="e4d")
            for c4 in range(4):
                nc.scalar.dma_start(
                    out=e4pk[20 * c4:20 * c4 + 16, :],
                    in_=ld[:, c4 * CHUNK:(c4 + 1) * CHUNK]
                        .rearrange("u (b j) -> u b j", j=1024))
                nc.scalar.dma_start(
                    out=e4pk[20 * c4 + 16:20 * c4 + 20, :],
                    in_=ld[0, c4 * CHUNK:(c4 + 1) * CHUNK]
                        .rearrange("(b j) -> b j", j=1024))

            # e4pk row 20c+4u+b, col j = lnx[u] at batch c*4096+1024b+j
            # (u=4 passthrough rows use lnx[0] as a finite filler; scale 0).
            # Packed [80,1024]: ACT cost is per free-dim element, so 4x
            # cheaper than [20,4096]. Unpacks into l4x via a DRAM bounce.
            # l4x rows 0..14: chunks 0-2; rows 32..36: chunk 3 (32-aligned).
            l3x = bpool.tile([128, CHUNK], F16, tag="l3x", bufs=1)
            nc.gpsimd.memset(l3x[:, :], 0.0)
            for b, ch in enumerate([0, 1, 3, 2]):
                nc.scalar.dma_start(
                    out=l3x[32 * b:32 * b + 20, :],
                    in_=ld[:, ch * CHUNK:(ch + 1) * CHUNK].unsqueeze(0)
                        .broadcast_to([5, V, CHUNK]))
            load_lrep(0)
            load_lrep(1)
            load_lrep(2)

            e2s = [None] * NCH
            PW = 512                   # pipeline piece width (1 PSUM bank)

            # ---------- pipeline stages for one column piece ----------
            def st_ps1(cc, pc, e1s, w=PW):
                ps1 = ppool.tile([128, PW], F32, tag="psA", bufs=4,
                                 name="ps1")[:, 0:w]
                for s in range((w + MMN - 1) // MMN):
                    scol = s * MMN
                    sw = min(MMN, w - scol)
                    for t in range(NT1):
                        nc.tensor.matmul(
                            ps1[:, scol:scol + sw],
                            g1t[:, 128 * t:128 * (t + 1)],
                            e1s[t][:, pc + scol:pc + scol + sw],
                            start=(t == 0), stop=(t == NT1 - 1))
                return ps1

            def st_x2(cc, pc, ps1, w=PW):
                e2 = e2s[cc]
                nc.vector.tensor_mul(
                    e2[:, pc:pc + w], e2[:, pc:pc + w], ps1[:])
                ps2 = ppool.tile([32, PW], F32, tag="psB", bufs=4,
                                 name="ps2")[:, 0:w]
                nc.tensor.matmul(
                    ps2[:, 0:w], g2t[:], e2[:, pc:pc + w],
                    start=True, stop=True)
                return ps2

            f2e = bpool.tile([32, CHUNK], F16, tag="f2e", bufs=1)

            def st_x3(cc, pc, ps2, w=PW):
                # X3 = E3 * f2
                if cc == 0:
                    # chunk 0: the l3x exp may not have landed yet (its DMA
                    # chain is long); drain f2 to SBUF so ps2 slots recycle
                    # immediately, and multiply later (see cc==2)
                    nc.vector.tensor_copy(f2e[:, pc:pc + w], ps2[:])
                else:
                    # fused drain of ps2 (in place into l3x rows)
                    nc.vector.tensor_mul(
                        l3x[32 * cc:32 * cc + 32, pc:pc + w],
                        l3x[32 * cc:32 * cc + 32, pc:pc + w], ps2[:])

            def early_tail(pc, w=PW):
                """Levels 3+4 for chunks 0-2 (l3x rows <96, l4x rows <15,
                out rows 0..2) — runs during the chunk-2 window."""
                ps3a = ppool.tile([32, PW], F32, tag="psA", bufs=4,
                                  name="ps3a")[:, 0:w]
                nc.tensor.matmul(ps3a[:, 0:w], g3at[:], l3x[:, pc:pc + w],
                                 start=True, stop=True)
                nc.vector.tensor_mul(l4x[0:15, pc:pc + w],
                                     l4x[0:15, pc:pc + w], ps3a[0:15, :])
                ps4a = ppool.tile([NCH, PW], F32, tag="psB", bufs=4,
                                  name="ps4a")[:, 0:w]
                nc.tensor.matmul(ps4a[:, 0:w], g4at[:], l4x[0:15, pc:pc + w],
                                 start=True, stop=True)
                nc.vector.tensor_copy(outsbA[0:3, pc:pc + w], ps4a[0:3, :])

            def st_ps3b(pc, w=PW):
                ps3b = ppool.tile([32, PW], F32, tag="psA", bufs=4,
                                  name="ps3b")[:, 0:w]
                nc.tensor.matmul(ps3b[:, 0:w], g3bt[:], l3x[:, pc:pc + w],
                                 start=True, stop=True)
                return ps3b

            def st_x4b(pc, ps3b, w=PW):
                nc.vector.tensor_mul(l4x[32:37, pc:pc + w],
                                     l4x[32:37, pc:pc + w], ps3b[0:5, :])
                ps4b = ppool.tile([1, PW], F32, tag="psB", bufs=4,
                                  name="ps4b")[:, 0:w]
                nc.tensor.matmul(ps4b[:, 0:w], g4bt[:], l4x[32:37, pc:pc + w],
                                 start=True, stop=True)
                return ps4b

            def st_out(pc, ps4b, tail, w=PW):
                if tail:
                    nc.scalar.copy(outsbB[0:1, pc:pc + w], ps4b[0:1, :])
                else:
                    nc.vector.tensor_copy(outsbB[0:1, pc:pc + w],
                                          ps4b[0:1, :])

            def phase12(cc, pc, e1s):
                """levels 1+2 for chunk cc, columns [pc, pc+PW)."""
                ps1 = st_ps1(cc, pc, e1s)
                ps2 = st_x2(cc, pc, ps1)
                st_x3(cc, pc, ps2)

            def tail_pipeline(pcs, e1s, cc):
                """Chunk-3 pieces (pc, w) pairs, DVE ops emitted in skewed
                wavefronts so the strict-FIFO vector queue never stalls at
                its head. Only the LATE (chunk-3) levels 3/4 run here."""
                n = len(pcs)
                ps1s = [st_ps1(cc, pc, e1s, w) for pc, w in pcs]
                ps2 = {}
                ps3 = {}
                ps4 = {}
                for d in range(n + 3):
                    for i, (pc, w) in reversed(list(enumerate(pcs))):
                        s = d - i
                        if s == 0:
                            ps2[i] = st_x2(cc, pc, ps1s[i], w)
                        elif s == 1:
                            # x3b: fused drain of ps2 into chunk-3 rows
                            nc.vector.tensor_mul(
                                l3x[96:128, pc:pc + w],
                                l3x[96:128, pc:pc + w], ps2[i][:, :])
                            ps3[i] = st_ps3b(pc, w)
                        elif s == 2:
                            ps4[i] = st_x4b(pc, ps3[i], w)
                        elif s == 3:
                            st_out(pc, ps4[i], tail=(pc >= 3072), w=w)

            outsbA = bpool.tile([3, CHUNK], F32, tag="outsbA", bufs=1)
            outsbB = bpool.tile([1, CHUNK], F32, tag="outsbB", bufs=1)

            # ---------- per-chunk: exps then levels 1+2 ----------
            # chunk 0: head split (1024/3072) so the first exp starts as
            # soon as the first lrep0 quarter lands; chunks 1-2 whole-chunk
            # exp calls; chunk 3: quarter-chunk calls with the late path
            # interleaved so the post-exp tail is short
            for cc in range(NCH):
                if cc == 2:
                    # deferred l3 mul for chunk 0 (fast fp16 SBUF mul)
                    nc.vector.tensor_mul(l3x[0:32, :], l3x[0:32, :],
                                         f2e[:, :])
                last = cc == NCH - 1
                e1s = [None] * NT1
                if cc == 0:
                    splits = [(0, 1024), (1024, CHUNK - 1024)]
                elif last:
                    splits = [(q * 1024, 1024) for q in range(4)]
                else:
                    splits = [(0, CHUNK)]
                for hh, (hc, w) in enumerate(splits):
                    for t in range(NT1):
                        if hh == 0:
                            e1s[t] = bpool.tile([128, CHUNK], F16, tag="e1",
                                                bufs=8, name=f"e1_{cc}_{t}")
                        nc.scalar.activation(
                            e1s[t][:, hc:hc + w], lreps[cc][:, hc:hc + w],
                            EXP, scale=sct1[:, t:t + 1])
                    if hh == 0:
                        e2 = bpool.tile([128, CHUNK], F16, tag="e2", bufs=3,
                                        name=f"e2_{cc}")
                        e2s[cc] = e2
                    nc.scalar.activation(e2s[cc][:, hc:hc + w],
                                         lreps[cc][:, hc:hc + w], EXP,
                                         scale=sct2[:, 0:1])
                    if last:
                        pieces = [(p * PW, PW) for p in
                                  range(hc // PW, (hc + w) // PW)]
                        tail_pipeline(pieces, e1s, cc)
                    else:
                        for p in range(hc // PW, (hc + w) // PW):
                            phase12(cc, p * PW, e1s)
                            if cc == 2:
                                early_tail(p * PW)
                if cc == 0:
                    # hoisted phase-B exps at the end of chunk 0 (their DMA
                    # chains have landed by now)
                    nc.scalar.activation(l3x[:], l3x[:], EXP,
                                         scale=sct3[:, 0:1])
                    nc.scalar.activation(e4pk[:], e4pk[:], EXP,
                                         scale=sct4[:, 0:1])
                    # unpack [80,1024] -> [20,4096] via DRAM bounce
                    # (partition-split SBUF source APs are unsupported)
                    nc.sync.dma_start(out=e4d[:, :], in_=e4pk[:, :])
                    nc.sync.dma_start(
                        out=l4x[0:15, :],
                        in_=e4d[0:60, :].rearrange("(m b) j -> m b j", b=4))
                    nc.sync.dma_start(
                        out=l4x[32:37, :],
                        in_=e4d[60:80, :].rearrange("(m b) j -> m b j", b=4))
                if cc == 2:
                    # out rows 0..2 complete once all early_tail copies land;
                    # this DMA fires mid-chunk-3, off the critical path
                    nc.sync.dma_start(
                        out=y[0:3 * CHUNK].rearrange("(c i) -> c i",
                                                     i=CHUNK),
                        in_=outsbA[:, :])

            nc.sync.dma_start(
                out=y[3 * CHUNK:4 * CHUNK].rearrange("(c i) -> c i",
                                                     i=CHUNK),
                in_=outsbB[:, :])

    nc.compile()
    return nc


def kernel(x, lam0, lam1, pow1, lam2, pow2, lam3, pow3, lam4, pow4):
    x = np.asarray(x, np.float32)
    consts = build_constants(
        np.asarray(lam0, np.float32), np.asarray(lam1, np.float32),
        np.asarray(pow1, np.float32), np.asarray(lam2, np.float32),
        np.asarray(pow2, np.float32), np.asarray(lam3, np.float32),
        np.asarray(pow3, np.float32), np.asarray(lam4, np.float32),
        np.asarray(pow4, np.float32))

    nc = build_bass()

    in_maps = []
    for k in range(M_CORES):
        shard = x[k * BS:(k + 1) * BS, :]
        m = {"xt": np.ascontiguousarray(shard.T)}
        m.update(consts)
        in_maps.append(m)

    from concourse.bass_utils import run_bass_kernel_spmd
    res = run_bass_kernel_spmd(nc, in_maps, list(range(M_CORES)))
    out = np.concatenate([res.results[k]["y"] for k in range(M_CORES)])
    return out[:, None].astype(np.float32)


if __name__ == "__main__":
    import reference
    inputs = {k: np.asarray(v) for k, v in reference.setup_inputs().items()}
    got = kernel(**inputs)
    exp = np.asarray(reference.reference(**inputs))
    err = np.abs(got - exp).max() / (np.abs(exp).max() + 1e-30)
    print("shape", got.shape, "relerr", err)
